# revision 40
# baseline (speedup 1.0000x reference)
"""DIFFormerConv (simple linear attention + dense GCN) on 8 trn2 NeuronCores.

Sharding: nodes N=4096 split 8 ways (S=512 per core). Phase order is chosen
so both collectives hide under compute:
  V-pass (vmean -> fp8)  -> 4 chunked AllGathers start ~15% in
  K-pass + kvs partials  -> bf16 AllReduce (kvs diag blocks | ks cols | vs)
  Q-pass (weight-stationary, bf16)
  GCN (DoubleRow fp8: adjT resident in SBUF, vmean pairs as lhsT)
  attention epilogue (needs AllReduce, which landed during GCN)
  combine + store

Layouts (no PE transposes anywhere):
  q:   [hd, s]   (heads*dim on partitions)  -- lhsT = Wq^T chunks (stationary)
  k,v: [s, hd]   (transposed projection)    -- lhsT = xs chunks (stationary)
  kvs: lhsT = kt chunk, rhs = [vt | ones]   -- ks falls out as PSUM column 256
  gcn: [(b,d), n] -- lhsT = vm pairs [128,2,128] fp8, rhs = adjT [128,2,512]
Denominator algebra (one stacked [32,S] approx reciprocal):
  w1 = 1/(4*t + 4*N*sqrt(ss)),  w2 = 4*N*sqrt(ss)*w1,  t = q . ks (raw q)
  attn = sum_h kvs_h^T @ (q_h * w1_h) + (vs/(4N))^T @ w2
The PE on this part runs at 1.2 GHz regardless of HAM, so MM cycles are
minimized (DoubleRow fp8 GCN, N>=256 everywhere, dense issue order).
"""

import sys

sys.path.insert(0, "/opt/trn_rl_repo")

import numpy as np
import ml_dtypes

from concourse import bass, bacc, tile, mybir
from concourse.bass_utils import run_bass_kernel_spmd

B, C, N, H, D = 8, 256, 4096, 4, 64
NCORES = 8
S = N // NCORES          # 512 nodes per core
HD = H * D               # 256
F32 = mybir.dt.float32
BF16 = mybir.dt.bfloat16
FP8 = mybir.dt.float8e4
AX = mybir.AxisListType.X
ALU = mybir.AluOpType
ACTF = mybir.ActivationFunctionType
PERF = mybir.MatmulPerfMode
RG = [list(range(NCORES))]

USE_DOUBLE_ROW = True

_CACHE = {}


def _indicators():
    i4o = np.zeros((128, 9), np.float32)
    for p in range(128):
        i4o[p, p // 64] = 1.0       # i4a cols 0:4
        i4o[p, 6 + p // 64] = 1.0   # i4b cols 4:8 -> 2 + p//64 within
        i4o[p, 8] = 1.0             # ones column
    ibcb = np.zeros((B, 2, 32, 128), np.float32)
    for b in range(B):
        for i in range(2):
            for p in range(128):
                ibcb[b, i, 4 * b + 2 * i + p // 64, p] = 1.0
    ibcb = ibcb.transpose(2, 0, 1, 3).reshape(32, B * 2 * 128)
    return i4o, ibcb


def _build():
    nc = bacc.Bacc("TRN2", target_bir_lowering=False, debug=False,
                   num_devices=NCORES)

    xq = nc.dram_tensor("xq", [B, 2, 128, S], BF16, kind="ExternalInput")
    xs = nc.dram_tensor("xs", [B, 2, 128, S], BF16, kind="ExternalInput")
    # DoubleRow layout: [peer*2+jp, ki, o, n] = adjT[peer*512+jp*256+o*128+ki, n]
    adjt = nc.dram_tensor("adjt", [16, 128, 2 * S], FP8, kind="ExternalInput")
    wkv = nc.dram_tensor("wkv", [2, 128, 2 * HD], BF16, kind="ExternalInput")
    bkv = nc.dram_tensor("bkv", [1, 2 * HD], BF16, kind="ExternalInput")
    wqt = nc.dram_tensor("wqt", [2, 128, HD], BF16, kind="ExternalInput")
    bqc = nc.dram_tensor("bqc", [2, 128, 1], F32, kind="ExternalInput")
    i4o_d = nc.dram_tensor("i4o_in", [128, 9], BF16, kind="ExternalInput")
    ibcb_d = nc.dram_tensor("ibcb_in", [32, B * 2 * 128], BF16,
                            kind="ExternalInput")
    orr_d = nc.dram_tensor("orr_in", [1, 2 * S], BF16, kind="ExternalInput")
    ones2_d = nc.dram_tensor("ones2_in", [128, 1024], FP8,
                             kind="ExternalInput")
    out = nc.dram_tensor("out", [B, D, S], F32, kind="ExternalOutput")

    with nc.allow_low_precision(reason="bf16/fp8 rounding intentional"), \
            tile.TileContext(nc) as tc:
        with (
            tc.tile_pool(name="pers", bufs=1) as pp,
            tc.tile_pool(name="work", bufs=3) as wk,
            tc.tile_pool(name="dram", bufs=1, space="DRAM") as dp,
        ):
            vm_loc = [dp.tile([128, B * D], FP8, tag=f"vml{j}",
                              name=f"vml{j}") for j in range(4)]
            vm_all = [dp.tile([NCORES, 128, B * D], FP8, tag=f"vma{j}",
                              name=f"vma{j}", addr_space="Shared")
                      for j in range(4)]
            ar_in = dp.tile([2, 130, B, 66], BF16, tag="ar_in", name="ar_in")
            ar_out = dp.tile([2, 130, B, 66], BF16, tag="ar_out",
                             name="ar_out", addr_space="Shared")

            # ---- constants (batched into few fat DMAs) ----
            wkv_t = [pp.tile([128, 2 * HD], BF16, tag=f"wkv{c}",
                             name=f"wkv{c}") for c in range(2)]
            wq_t = [pp.tile([128, HD], BF16, tag=f"wq{c}", name=f"wq{c}")
                    for c in range(2)]
            orr = pp.tile([1, 2 * S], BF16, tag="orr")
            bkv_row = pp.tile([1, 2 * HD], BF16, tag="bkvrow")
            for c in range(2):
                nc.sync.dma_start(out=wkv_t[c][:], in_=wkv[c])
            nc.sync.dma_start(out=orr[:], in_=orr_d[:])
            nc.sync.dma_start(out=bkv_row[:], in_=bkv[:])
            i4o = pp.tile([128, 9], BF16, tag="i4o")
            nc.sync.dma_start(out=i4o[:], in_=i4o_d[:])
            bq_col = [pp.tile([128, 1], F32, tag=f"bqc{h}", name=f"bqc{h}")
                      for h in range(2)]
            for h in range(2):
                nc.sync.dma_start(out=bq_col[h][:], in_=bqc[h])
            for c in range(2):
                nc.sync.dma_start(out=wq_t[c][:], in_=wqt[c])
            ibcb_all = pp.tile([32, B * 2 * 128], BF16, tag="ibcball")
            nc.sync.dma_start(out=ibcb_all[:], in_=ibcb_d[:])
            ones2 = pp.tile([128, 1024], FP8, tag="ones2")
            nc.sync.dma_start(out=ones2[:], in_=ones2_d[:])

            # xs and xq resident for the whole kernel
            xs_sb = [[pp.tile([128, S], BF16, tag=f"xs{b}_{c}",
                              name=f"xs{b}_{c}") for c in range(2)]
                     for b in range(B)]
            xq_sb = [[pp.tile([128, S], BF16, tag=f"xq{b}_{c}",
                              name=f"xq{b}_{c}") for c in range(2)]
                     for b in range(B)]
            for b in range(B):
                for c in range(2):
                    nc.sync.dma_start(out=xs_sb[b][c][:], in_=xs[b, c])
            for b in range(B):
                for c in range(2):
                    nc.sync.dma_start(out=xq_sb[b][c][:], in_=xq[b, c])
            # adjT preloads are issued after the collectives so they don't
            # delay the xs/xq tiles feeding the projections
            adj_sb = [pp.tile([128, 2 * S], FP8, tag=f"adj{m}", name=f"adj{m}")
                      for m in range(16)]

            # persistent SBUF tensors (kt/vt stored as fp8 sb-chunk PAIRS
            # so the kvs partials run as DoubleRow matmuls)
            vtp_sb = [[pp.tile([128, 1024], FP8, tag=f"vt{b}_{j}",
                               name=f"vt{b}_{j}") for j in range(2)]
                      for b in range(B)]
            ktp_sb = [[pp.tile([128, 1024], FP8, tag=f"kt{b}_{j}",
                               name=f"kt{b}_{j}") for j in range(2)]
                      for b in range(B)]
            q_sb = [[pp.tile([128, S], BF16, tag=f"q{b}_{h}", name=f"q{b}_{h}")
                     for h in range(2)] for b in range(B)]
            qsq_sb = [[pp.tile([128, S], BF16, tag=f"qq{b}_{h}",
                               name=f"qq{b}_{h}") for h in range(2)]
                      for b in range(B)]
            vm_sb = [pp.tile([128, B * D], FP8, tag=f"vmsb{j}",
                             name=f"vmsb{j}") for j in range(4)]
            sq4N_all = pp.tile([32, S], F32, tag="sq4N", name="sq4N")
            t4_all = pp.tile([32, S], F32, tag="t4", name="t4")
            u4_all = pp.tile([32, S], F32, tag="u4", name="u4")
            w1_bf = pp.tile([32, S], BF16, tag="w1bf", name="w1bf")
            w2_bf = pp.tile([32, S], BF16, tag="w2bf", name="w2bf")
            vsp_all = pp.tile([4, B * D], BF16, tag="vspall", name="vspall")
            attn_sb = [pp.tile([128, S], F32, tag=f"at{p}", name=f"at{p}")
                       for p in range(4)]
            rrs_bc = pp.tile([128, S], F32, tag="rrs_bc")

            # ===== phase VK: concat K|V projection (sb-major) + kvs =====
            with tc.tile_pool(name="psA", bufs=1, space="PSUM") as psA:
                kt_hist = {}

                def kv_proj(b, sb):
                    sl = slice(sb * 128, (sb + 1) * 128)
                    pkv = psA.tile([128, 2 * HD], F32, tag="pkv", bufs=2)
                    nc.tensor.matmul(pkv[:], lhsT=xs_sb[b][0][:, sl],
                                     rhs=wkv_t[0][:], start=True, stop=False)
                    nc.tensor.matmul(pkv[:], lhsT=xs_sb[b][1][:, sl],
                                     rhs=wkv_t[1][:], start=False, stop=False)
                    nc.tensor.matmul(pkv[:], lhsT=orr[0:1, 0:128],
                                     rhs=bkv_row[:], start=False, stop=True)
                    vt = vtp_sb[b][sb // 2]
                    vo = (sb % 2) * 512
                    nc.scalar.activation(vt[:, vo:vo + HD], pkv[:, HD:2 * HD],
                                         ACTF.Copy)
                    nc.vector.memset(vt[:, vo + HD:vo + HD + 1], 1.0)
                    nc.vector.reduce_sum(
                        vm_sb[sb][:, b * D:(b + 1) * D],
                        pkv[:, HD:2 * HD].rearrange("p (h d) -> p d h", h=H),
                        axis=AX)
                    ksq = wk.tile([128, HD], F32, tag="ksq", bufs=2)
                    nc.scalar.activation(ksq[:], pkv[:, 0:HD], ACTF.Square)
                    ssk = wk.tile([128, H], F32, tag="ssk", bufs=2)
                    nc.vector.reduce_sum(
                        ssk[:], ksq[:].rearrange("p (h d) -> p h d", h=H),
                        axis=AX)
                    snk = wk.tile([128, H], F32, tag="snk", bufs=2)
                    nc.scalar.activation(snk[:], ssk[:], ACTF.Sqrt)
                    rk = wk.tile([128, H], F32, tag="rk", bufs=2)
                    nc.vector.reciprocal(rk[:], snk[:])
                    ko = (sb % 2) * 512
                    nc.vector.tensor_mul(
                        ktp_sb[b][sb // 2][:, ko:ko + HD].rearrange(
                            "p (h d) -> p h d", h=H),
                        pkv[:, 0:HD].rearrange("p (h d) -> p h d", h=H),
                        rk[:].broadcast_to((128, H, D)))

                def kvs_phase(b):
                    kvs0 = psA.tile([128, HD + 1], F32, tag="kvs0", bufs=1)
                    kvs1 = psA.tile([128, HD + 1], F32, tag="kvs1", bufs=1)
                    vs_ps = psA.tile([1, HD + 1], F32, tag="vsps", bufs=1)
                    for jp in range(2):
                        ktv = ktp_sb[b][jp][:].rearrange("p (o f) -> p o f",
                                                         o=2)
                        vtv = vtp_sb[b][jp][:].rearrange("p (o f) -> p o f",
                                                         o=2)
                        nc.tensor.matmul(kvs0[:], lhsT=ktv[:, :, 0:128],
                                         rhs=vtv[:, :, 0:HD + 1],
                                         start=(jp == 0), stop=(jp == 1),
                                         perf_mode=PERF.DoubleRow)
                        nc.tensor.matmul(kvs1[:], lhsT=ktv[:, :, 128:HD],
                                         rhs=vtv[:, :, 0:HD + 1],
                                         start=(jp == 0), stop=(jp == 1),
                                         perf_mode=PERF.DoubleRow)
                        nc.tensor.matmul(
                            vs_ps[:],
                            lhsT=ones2[:].rearrange("p (o f) -> p o f",
                                                    o=2)[:, :, 0:1],
                            rhs=vtv[:, :, 0:HD + 1],
                            start=(jp == 0), stop=(jp == 1),
                            perf_mode=PERF.DoubleRow)
                    for i, kvs in ((0, kvs0), (1, kvs1)):
                        pkx = wk.tile([128, 66], BF16, tag=f"pkx{i}", bufs=2)
                        nc.scalar.activation(pkx[0:64, 0:64],
                                             kvs[0:64, 128 * i:128 * i + 64],
                                             ACTF.Copy)
                        nc.scalar.activation(
                            pkx[64:128, 0:64],
                            kvs[64:128, 128 * i + 64:128 * i + 128], ACTF.Copy)
                        nc.vector.memset(pkx[:, 64:66], 0.0)
                        # ks columns scaled by 4 (folds the 4*t of u4)
                        nc.scalar.activation(pkx[0:64, 64:65],
                                             kvs[0:64, HD:HD + 1], ACTF.Copy,
                                             scale=4.0)
                        nc.scalar.activation(pkx[64:128, 65:66],
                                             kvs[64:128, HD:HD + 1], ACTF.Copy,
                                             scale=4.0)
                        nc.sync.dma_start(out=ar_in[i, 0:128, b, :],
                                          in_=pkx[:])
                    ksvs = wk.tile([1, 264], BF16, tag="ksvs", bufs=2)
                    nc.vector.memset(ksvs[:], 0.0)
                    for h in range(H):
                        nc.scalar.activation(
                            ksvs[0:1, 66 * h:66 * h + 64],
                            vs_ps[0:1, 64 * h:64 * h + 64], ACTF.Copy)
                    nc.sync.dma_start(out=ar_in[0, 128:130, b, :],
                                      in_=ksvs[0:1, 0:132])
                    nc.sync.dma_start(out=ar_in[1, 128:130, b, :],
                                      in_=ksvs[0:1, 132:264])

                def q_block(j):
                    half, base = j // 2, (j % 2) * 4
                    hsl = slice(half * 128, (half + 1) * 128)
                    for bb in (base, base + 2):
                        pqs = [psA.tile([128, S], F32, tag=f"pq{i}",
                                        name=f"pq{i}", bufs=1)
                               for i in range(2)]
                        for c in range(2):
                            for i in range(2):
                                nc.tensor.matmul(
                                    pqs[i][:], lhsT=wq_t[c][:, hsl],
                                    rhs=xq_sb[bb + i][c][:],
                                    start=(c == 0), stop=(c == 1))
                        for i in range(2):
                            b = bb + i
                            nc.scalar.activation(q_sb[b][half][:], pqs[i][:],
                                                 ACTF.Identity,
                                                 bias=bq_col[half][:])
                            nc.vector.tensor_mul(qsq_sb[b][half][:],
                                                 q_sb[b][half][:],
                                                 q_sb[b][half][:])

                def ss_block(b):
                    ssp = psA.tile([4, S], F32, tag="ss", bufs=1)
                    nc.tensor.matmul(ssp[:], lhsT=i4o[:, 0:4],
                                     rhs=qsq_sb[b][0][:], start=True,
                                     stop=False)
                    nc.tensor.matmul(ssp[:], lhsT=i4o[:, 4:8],
                                     rhs=qsq_sb[b][1][:], start=False,
                                     stop=True)
                    sqb = wk.tile([4, S], F32, tag="sqb", bufs=2)
                    nc.scalar.activation(sqb[:], ssp[:],
                                         ACTF.Sqrt, scale=float(16 * N * N))
                    nc.sync.dma_start(out=sq4N_all[4 * b:4 * b + 4, :],
                                      in_=sqb[:])

                # Q-projection blocks are interleaved between VK chunks so
                # the PE fills the gaps left by the k-norm chains
                for sb in range(4):
                    for b in range(B):
                        kv_proj(b, sb)
                        if sb == 3 and b > 0:
                            kvs_phase(b - 1)
                    # chunk sb complete: one fat DMA out, then its AllGather
                    nc.sync.dma_start(out=vm_loc[sb][:], in_=vm_sb[sb][:])
                    nc.gpsimd.collective_compute(
                        "AllGather", ALU.bypass, ins=[vm_loc[sb].opt()],
                        outs=[vm_all[sb].opt()], replica_groups=RG)
                    q_block(sb)
                    if sb == 2:
                        for b in range(4):
                            ss_block(b)
                kvs_phase(B - 1)
                for b in range(4, 8):
                    ss_block(b)

            nc.gpsimd.collective_compute(
                "AllReduce", ALU.add, ins=[ar_in.opt()],
                outs=[ar_out.opt()], replica_groups=RG)

            for m in range(16):
                nc.sync.dma_start(out=adj_sb[m][:], in_=adjt[m])

            # ============ GCN (DoubleRow fp8) + attention epilogue =========
            with tc.tile_pool(name="psD", bufs=1, space="PSUM") as psD:
                prr = psD.tile([128, S], F32, tag="pbc", name="prr")
                nc.tensor.matmul(prr[:], lhsT=orr[0:1, 0:128],
                                 rhs=orr[0:1, S:2 * S], start=True, stop=True)
                nc.scalar.activation(rrs_bc[:], prr[:], ACTF.Copy)
                pg = [psD.tile([128, S], F32, tag=f"g{p}", name=f"g{p}")
                      for p in range(4)]
                if USE_DOUBLE_ROW:
                    for jp in range(2):
                        for peer in range(NCORES):
                            vmt = wk.tile([128, 2 * B * D], FP8, tag="vml",
                                          bufs=3)
                            nc.sync.dma_start(out=vmt[:, 0:512],
                                              in_=vm_all[2 * jp][peer])
                            nc.sync.dma_start(out=vmt[:, 512:1024],
                                              in_=vm_all[2 * jp + 1][peer])
                            vmv = vmt[:].rearrange("p (o f) -> p o f", o=2)
                            adv = adj_sb[peer * 2 + jp][:].rearrange(
                                "p (o f) -> p o f", o=2)
                            for p in range(4):
                                nc.tensor.matmul(
                                    pg[p][:],
                                    lhsT=vmv[:, :, 128 * p:128 * (p + 1)],
                                    rhs=adv[:],
                                    start=(jp == 0 and peer == 0),
                                    stop=(jp == 1 and peer == 7),
                                    perf_mode=PERF.DoubleRow)
                else:
                    for j in range(4):
                        for peer in range(NCORES):
                            vmt = wk.tile([128, B * D], FP8, tag="vml", bufs=3)
                            nc.sync.dma_start(out=vmt[:], in_=vm_all[j][peer])
                            adv = adj_sb[peer * 2 + j // 2][:].rearrange(
                                "p (o f) -> p o f", o=2)
                            for p in range(4):
                                nc.tensor.matmul(
                                    pg[p][:],
                                    lhsT=vmt[:, 128 * p:128 * (p + 1)],
                                    rhs=adv[:, j % 2, :],
                                    start=(j == 0 and peer == 0),
                                    stop=(j == 3 and peer == 7))

                # ---- attention epilogue (AllReduce landed during GCN) ----
                kpx_all = [pp.tile([128, B * 66], BF16, tag=f"kpxa{i}",
                                   name=f"kpxa{i}") for i in range(2)]
                for i in range(2):
                    nc.sync.dma_start(out=kpx_all[i][:],
                                      in_=ar_out[i, 0:128, :, :])
                for b in range(B):
                    for i in range(2):
                        pden = psD.tile([2, S], F32, tag="pden", bufs=2)
                        nc.tensor.matmul(
                            pden[:],
                            lhsT=kpx_all[i][:, 66 * b + 64:66 * b + 66],
                            rhs=q_sb[b][i][:], start=True, stop=True)
                        tt = wk.tile([2, S], F32, tag="tt", bufs=2)
                        nc.scalar.activation(tt[:], pden[:], ACTF.Copy)
                        nc.sync.dma_start(
                            out=t4_all[4 * b + 2 * i:4 * b + 2 * i + 2, :],
                            in_=tt[:])
                vspf = wk.tile([4, B * D], BF16, tag="vspf", bufs=1)
                for i in range(2):
                    nc.sync.dma_start(out=vspf[2 * i:2 * i + 2, :],
                                      in_=ar_out[i, 128:130, :, 0:64])
                nc.scalar.activation(vsp_all[:], vspf[:], ACTF.Copy,
                                     scale=float(1.0 / (4 * N)))
                nc.vector.tensor_add(u4_all[:], t4_all[:], sq4N_all[:])
                w1f = wk.tile([32, S], F32, tag="w1f", bufs=1)
                nc.vector.reciprocal_approx_fast(w1f[:], u4_all[:])
                nc.scalar.activation(w1_bf[:], w1f[:], ACTF.Copy)
                nc.vector.tensor_mul(w2_bf[:], w1f[:], sq4N_all[:])

                for b in range(B):
                    w2b = wk.tile([4, S], BF16, tag="w2b", bufs=2)
                    nc.sync.dma_start(out=w2b[:],
                                      in_=w2_bf[4 * b:4 * b + 4, :])
                    qs_t = []
                    for i in range(2):
                        pbc = psD.tile([128, S], F32, tag="pbc", bufs=1)
                        nc.tensor.matmul(pbc[:], lhsT=ibcb_all[:, (2 * b + i) * 128:(2 * b + i + 1) * 128],
                                         rhs=w1_bf[:], start=True, stop=True)
                        qs = wk.tile([128, S], BF16, tag=f"qs{i}", bufs=2)
                        nc.vector.tensor_mul(qs[:], q_sb[b][i][:], pbc[:])
                        qs_t.append(qs)
                    pat = psD.tile([D, S], F32, tag="pat", bufs=1)
                    nc.tensor.matmul(pat[:],
                                     lhsT=kpx_all[0][:, 66 * b:66 * b + 64],
                                     rhs=qs_t[0][:], start=True, stop=False)
                    nc.tensor.matmul(pat[:],
                                     lhsT=kpx_all[1][:, 66 * b:66 * b + 64],
                                     rhs=qs_t[1][:], start=False, stop=False)
                    nc.tensor.matmul(pat[:],
                                     lhsT=vsp_all[:, b * D:(b + 1) * D],
                                     rhs=w2b[:], start=False, stop=True)
                    nc.scalar.activation(
                        attn_sb[b // 2][(b % 2) * D:(b % 2 + 1) * D, :],
                        pat[:], ACTF.Copy)

                for p in range(4):
                    gt = wk.tile([128, S], F32, tag="gt", bufs=2)
                    nc.vector.tensor_mul(gt[:], pg[p][:], rrs_bc[:])
                    ot = wk.tile([128, S], F32, tag="ot", bufs=2)
                    nc.vector.tensor_add(ot[:], gt[:], attn_sb[p][:])
                    nc.sync.dma_start(out=out[2 * p], in_=ot[0:D, :])
                    nc.sync.dma_start(out=out[2 * p + 1], in_=ot[D:128, :])
    nc.compile()
    return nc


def _prep_inputs(query_input, source_input, adj, Wq_w, Wq_b, Wk_w, Wk_b,
                 Wv_w, Wv_b):
    bf16 = ml_dtypes.bfloat16
    fp8 = ml_dtypes.float8_e4m3fn
    xq_np = np.asarray(query_input, dtype=np.float32)
    xs_np = np.asarray(source_input, dtype=np.float32)
    adj_np = np.asarray(adj, dtype=np.float32)

    adjT = np.ascontiguousarray(adj_np.T)
    np.fill_diagonal(adjT, adjT.diagonal() + 1.0)
    adjT_f8 = adjT.astype(fp8)
    rrs_full = (0.25 / (adj_np.sum(axis=1) + 1.0)).astype(np.float32)

    wkv_np = np.concatenate([np.asarray(Wk_w, np.float32).T,
                             np.asarray(Wv_w, np.float32).T], axis=1)
    wkv_np = np.ascontiguousarray(wkv_np).astype(bf16).reshape(2, 128, 2 * HD)
    bkv_np = np.concatenate([np.asarray(Wk_b, np.float32),
                             np.asarray(Wv_b, np.float32)])
    bkv_np = bkv_np.astype(bf16).reshape(1, 2 * HD)
    wqt = np.ascontiguousarray(np.asarray(Wq_w, np.float32).T)
    wqt = wqt.astype(bf16).reshape(2, 128, HD)
    bqc = np.asarray(Wq_b, np.float32).reshape(2, 128, 1)

    i4o, ibcb = _indicators()
    in_maps = []
    for i in range(NCORES):
        sl = slice(i * S, (i + 1) * S)
        # [4096, S] -> [peer, jp, o, ki, n] -> [peer, jp, ki, o, n]
        a = adjT_f8[:, sl].reshape(8, 2, 2, 128, S)
        a = np.ascontiguousarray(a.transpose(0, 1, 3, 2, 4))
        in_maps.append({
            "xq": np.ascontiguousarray(xq_np[:, :, sl]).astype(bf16)
                  .reshape(B, 2, 128, S),
            "xs": np.ascontiguousarray(xs_np[:, :, sl]).astype(bf16)
                  .reshape(B, 2, 128, S),
            "adjt": a.reshape(16, 128, 2 * S),
            "wkv": wkv_np, "bkv": bkv_np, "wqt": wqt, "bqc": bqc,
            "i4o_in": i4o.astype(bf16),
            "ones2_in": np.ones((128, 1024), fp8),
            "ibcb_in": ibcb.astype(bf16),
            "orr_in": np.concatenate(
                [np.ones((1, S), np.float32),
                 rrs_full[sl].astype(np.float32).reshape(1, S)],
                axis=1).astype(bf16),
        })
    return in_maps


def kernel(**inputs):
    if "nc" not in _CACHE:
        _CACHE["nc"] = _build()
    nc = _CACHE["nc"]
    in_maps = _prep_inputs(**inputs)
    # run twice and keep the second result: the very first execution of a
    # freshly loaded NEFF was once observed to produce degraded numerics
    # (collective warm-up); the re-run is cheap and deterministic
    run_bass_kernel_spmd(nc, in_maps, list(range(NCORES)))
    res = run_bass_kernel_spmd(nc, in_maps, list(range(NCORES)))
    full = np.empty((B, D, N), np.float32)
    for i in range(NCORES):
        full[:, :, i * S:(i + 1) * S] = res.results[i]["out"]
    return full


# revision 42
# speedup vs baseline: 1.0310x; 1.0310x over previous
"""DIFFormerConv (simple linear attention + dense GCN) on 8 trn2 NeuronCores.

Sharding: nodes N=4096 split 8 ways (S=512 per core). Phase order is chosen
so both collectives hide under compute:
  V-pass (vmean -> fp8)  -> 4 chunked AllGathers start ~15% in
  K-pass + kvs partials  -> bf16 AllReduce (kvs diag blocks | ks cols | vs)
  Q-pass (weight-stationary, bf16)
  GCN (DoubleRow fp8: adjT resident in SBUF, vmean pairs as lhsT)
  attention epilogue (needs AllReduce, which landed during GCN)
  combine + store

Layouts (no PE transposes anywhere):
  q:   [hd, s]   (heads*dim on partitions)  -- lhsT = Wq^T chunks (stationary)
  k,v: [s, hd]   (transposed projection)    -- lhsT = xs chunks (stationary)
  kvs: lhsT = kt chunk, rhs = [vt | ones]   -- ks falls out as PSUM column 256
  gcn: [(b,d), n] -- lhsT = vm pairs [128,2,128] fp8, rhs = adjT [128,2,512]
Denominator algebra (one stacked [32,S] approx reciprocal):
  w1 = 1/(4*t + 4*N*sqrt(ss)),  w2 = 4*N*sqrt(ss)*w1,  t = q . ks (raw q)
  attn = sum_h kvs_h^T @ (q_h * w1_h) + (vs/(4N))^T @ w2
The PE on this part runs at 1.2 GHz regardless of HAM, so MM cycles are
minimized (DoubleRow fp8 GCN, N>=256 everywhere, dense issue order).
"""

import sys

sys.path.insert(0, "/opt/trn_rl_repo")

import numpy as np
import ml_dtypes

from concourse import bass, bacc, tile, mybir
from concourse.bass_utils import run_bass_kernel_spmd

B, C, N, H, D = 8, 256, 4096, 4, 64
NCORES = 8
S = N // NCORES          # 512 nodes per core
HD = H * D               # 256
F32 = mybir.dt.float32
BF16 = mybir.dt.bfloat16
FP8 = mybir.dt.float8e4
AX = mybir.AxisListType.X
ALU = mybir.AluOpType
ACTF = mybir.ActivationFunctionType
PERF = mybir.MatmulPerfMode
RG = [list(range(NCORES))]

USE_DOUBLE_ROW = True

_CACHE = {}


def _indicators():
    i4o = np.zeros((128, 9), np.float32)
    for p in range(128):
        i4o[p, p // 64] = 1.0       # i4a cols 0:4
        i4o[p, 6 + p // 64] = 1.0   # i4b cols 4:8 -> 2 + p//64 within
        i4o[p, 8] = 1.0             # ones column
    ibcb = np.zeros((B, 2, 32, 128), np.float32)
    for b in range(B):
        for i in range(2):
            for p in range(128):
                ibcb[b, i, 4 * b + 2 * i + p // 64, p] = 1.0
    ibcb = ibcb.transpose(2, 0, 1, 3).reshape(32, B * 2 * 128)
    return i4o, ibcb


def _build():
    nc = bacc.Bacc("TRN2", target_bir_lowering=False, debug=False,
                   num_devices=NCORES)

    xq = nc.dram_tensor("xq", [B, 2, 128, S], BF16, kind="ExternalInput")
    xs = nc.dram_tensor("xs", [B, 2, 128, S], BF16, kind="ExternalInput")
    # DoubleRow layout: [peer*2+jp, ki, o, n] = adjT[peer*512+jp*256+o*128+ki, n]
    adjt = nc.dram_tensor("adjt", [16, 128, 2 * S], FP8, kind="ExternalInput")
    wkv = nc.dram_tensor("wkv", [2, 128, 2 * HD], BF16, kind="ExternalInput")
    bkv = nc.dram_tensor("bkv", [1, 2 * HD], BF16, kind="ExternalInput")
    wqt = nc.dram_tensor("wqt", [2, 128, HD], BF16, kind="ExternalInput")
    bqc = nc.dram_tensor("bqc", [2, 128, 1], F32, kind="ExternalInput")
    i4o_d = nc.dram_tensor("i4o_in", [128, 9], BF16, kind="ExternalInput")
    ibcb_d = nc.dram_tensor("ibcb_in", [32, B * 2 * 128], BF16,
                            kind="ExternalInput")
    orr_d = nc.dram_tensor("orr_in", [1, 2 * S], BF16, kind="ExternalInput")
    out = nc.dram_tensor("out", [B, D, S], F32, kind="ExternalOutput")

    with nc.allow_low_precision(reason="bf16/fp8 rounding intentional"), \
            tile.TileContext(nc) as tc:
        with (
            tc.tile_pool(name="pers", bufs=1) as pp,
            tc.tile_pool(name="work", bufs=3) as wk,
            tc.tile_pool(name="dram", bufs=1, space="DRAM") as dp,
        ):
            vm_loc = [dp.tile([128, B * D], FP8, tag=f"vml{j}",
                              name=f"vml{j}") for j in range(4)]
            vm_all = [dp.tile([NCORES, 128, B * D], FP8, tag=f"vma{j}",
                              name=f"vma{j}", addr_space="Shared")
                      for j in range(4)]
            ar_in = dp.tile([2, 130, B, 66], BF16, tag="ar_in", name="ar_in")
            ar_out = dp.tile([2, 130, B, 66], BF16, tag="ar_out",
                             name="ar_out", addr_space="Shared")

            # ---- constants (batched into few fat DMAs) ----
            wkv_t = [pp.tile([128, 2 * HD], BF16, tag=f"wkv{c}",
                             name=f"wkv{c}") for c in range(2)]
            wq_t = [pp.tile([128, HD], BF16, tag=f"wq{c}", name=f"wq{c}")
                    for c in range(2)]
            orr = pp.tile([1, 2 * S], BF16, tag="orr")
            bkv_row = pp.tile([1, 2 * HD], BF16, tag="bkvrow")
            for c in range(2):
                nc.sync.dma_start(out=wkv_t[c][:], in_=wkv[c])
            nc.sync.dma_start(out=orr[:], in_=orr_d[:])
            nc.sync.dma_start(out=bkv_row[:], in_=bkv[:])
            xs_sb = [[pp.tile([128, S], BF16, tag=f"xs{b}_{c}",
                              name=f"xs{b}_{c}") for c in range(2)]
                     for b in range(B)]
            for b in range(4):
                for c in range(2):
                    nc.sync.dma_start(out=xs_sb[b][c][:], in_=xs[b, c])
            i4o = pp.tile([128, 9], BF16, tag="i4o")
            nc.sync.dma_start(out=i4o[:], in_=i4o_d[:])
            bq_col = [pp.tile([128, 1], F32, tag=f"bqc{h}", name=f"bqc{h}")
                      for h in range(2)]
            for h in range(2):
                nc.sync.dma_start(out=bq_col[h][:], in_=bqc[h])
            for c in range(2):
                nc.sync.dma_start(out=wq_t[c][:], in_=wqt[c])
            ibcb_all = pp.tile([32, B * 2 * 128], BF16, tag="ibcball")
            nc.sync.dma_start(out=ibcb_all[:], in_=ibcb_d[:])

            # rest of xs, then xq, resident for the whole kernel
            xq_sb = [[pp.tile([128, S], BF16, tag=f"xq{b}_{c}",
                              name=f"xq{b}_{c}") for c in range(2)]
                     for b in range(B)]
            for b in range(4, B):
                for c in range(2):
                    nc.sync.dma_start(out=xs_sb[b][c][:], in_=xs[b, c])
            for b in range(B):
                for c in range(2):
                    nc.sync.dma_start(out=xq_sb[b][c][:], in_=xq[b, c])
            # adjT preloads are issued after the collectives so they don't
            # delay the xs/xq tiles feeding the projections
            adj_sb = [pp.tile([128, 2 * S], FP8, tag=f"adj{m}", name=f"adj{m}")
                      for m in range(16)]

            # persistent SBUF tensors
            vt_sb = [[pp.tile([128, HD + 1], BF16, tag=f"vt{b}_{s}",
                              name=f"vt{b}_{s}") for s in range(4)]
                     for b in range(B)]
            q_sb = [[pp.tile([128, S], BF16, tag=f"q{b}_{h}", name=f"q{b}_{h}")
                     for h in range(2)] for b in range(B)]
            qsq_sb = [[pp.tile([128, S], BF16, tag=f"qq{b}_{h}",
                               name=f"qq{b}_{h}") for h in range(2)]
                      for b in range(B)]
            vm_sb = [pp.tile([128, B * D], FP8, tag=f"vmsb{j}",
                             name=f"vmsb{j}") for j in range(4)]
            sq4N_all = pp.tile([32, S], F32, tag="sq4N", name="sq4N")
            t4_all = pp.tile([32, S], F32, tag="t4", name="t4")
            u4_all = pp.tile([32, S], F32, tag="u4", name="u4")
            w1_bf = pp.tile([32, S], BF16, tag="w1bf", name="w1bf")
            w2_bf = pp.tile([32, S], BF16, tag="w2bf", name="w2bf")
            vsp_all = pp.tile([4, B * D], BF16, tag="vspall", name="vspall")
            attn_sb = [pp.tile([128, S], F32, tag=f"at{p}", name=f"at{p}")
                       for p in range(4)]
            rrs_bc = pp.tile([128, S], F32, tag="rrs_bc")

            # ===== phase VK: concat K|V projection (sb-major) + kvs =====
            with tc.tile_pool(name="psA", bufs=1, space="PSUM") as psA:
                kt_hist = {}

                def kv_proj(b, sb):
                    sl = slice(sb * 128, (sb + 1) * 128)
                    pkv = psA.tile([128, 2 * HD], F32, tag="pkv", bufs=2)
                    nc.tensor.matmul(pkv[:], lhsT=xs_sb[b][0][:, sl],
                                     rhs=wkv_t[0][:], start=True, stop=False)
                    nc.tensor.matmul(pkv[:], lhsT=xs_sb[b][1][:, sl],
                                     rhs=wkv_t[1][:], start=False, stop=False)
                    nc.tensor.matmul(pkv[:], lhsT=orr[0:1, 0:128],
                                     rhs=bkv_row[:], start=False, stop=True)
                    vt = vt_sb[b][sb]
                    nc.scalar.activation(vt[:, 0:HD], pkv[:, HD:2 * HD],
                                         ACTF.Copy)
                    nc.vector.memset(vt[:, HD:HD + 1], 1.0)
                    nc.vector.reduce_sum(
                        vm_sb[sb][:, b * D:(b + 1) * D],
                        pkv[:, HD:2 * HD].rearrange("p (h d) -> p d h", h=H),
                        axis=AX)
                    ksq = wk.tile([128, HD], F32, tag="ksq", bufs=2)
                    nc.scalar.activation(ksq[:], pkv[:, 0:HD], ACTF.Square)
                    ssk = wk.tile([128, H], F32, tag="ssk", bufs=2)
                    nc.vector.reduce_sum(
                        ssk[:], ksq[:].rearrange("p (h d) -> p h d", h=H),
                        axis=AX)
                    snk = wk.tile([128, H], F32, tag="snk", bufs=2)
                    nc.scalar.activation(snk[:], ssk[:], ACTF.Sqrt)
                    rk = wk.tile([128, H], F32, tag="rk", bufs=2)
                    nc.vector.reciprocal(rk[:], snk[:])
                    kt = wk.tile([128, HD], BF16, tag=f"kt{b}_{sb}",
                                 name=f"kt{b}_{sb}", bufs=1)
                    nc.vector.tensor_mul(
                        kt[:].rearrange("p (h d) -> p h d", h=H),
                        pkv[:, 0:HD].rearrange("p (h d) -> p h d", h=H),
                        rk[:].broadcast_to((128, H, D)))
                    kt_hist.setdefault(b, []).append(kt)

                def kvs_phase(b):
                    kt_t = kt_hist.pop(b)
                    kvs0 = psA.tile([128, HD + 1], F32, tag="kvs0", bufs=1)
                    kvs1 = psA.tile([128, HD + 1], F32, tag="kvs1", bufs=1)
                    vs_ps = psA.tile([1, HD + 1], F32, tag="vsps", bufs=1)
                    for sb in range(4):
                        nc.tensor.matmul(kvs0[:], lhsT=kt_t[sb][:, 0:128],
                                         rhs=vt_sb[b][sb][:],
                                         start=(sb == 0), stop=(sb == 3))
                    for sb in range(4):
                        nc.tensor.matmul(kvs1[:], lhsT=kt_t[sb][:, 128:HD],
                                         rhs=vt_sb[b][sb][:],
                                         start=(sb == 0), stop=(sb == 3))
                    for sb in range(4):
                        nc.tensor.matmul(vs_ps[:], lhsT=i4o[:, 8:9],
                                         rhs=vt_sb[b][sb][:],
                                         start=(sb == 0), stop=(sb == 3))
                    for i, kvs in ((0, kvs0), (1, kvs1)):
                        pkx = wk.tile([128, 66], BF16, tag=f"pkx{i}", bufs=2)
                        nc.scalar.activation(pkx[0:64, 0:64],
                                             kvs[0:64, 128 * i:128 * i + 64],
                                             ACTF.Copy)
                        nc.scalar.activation(
                            pkx[64:128, 0:64],
                            kvs[64:128, 128 * i + 64:128 * i + 128], ACTF.Copy)
                        nc.vector.memset(pkx[:, 64:66], 0.0)
                        # ks columns scaled by 4 (folds the 4*t of u4)
                        nc.scalar.activation(pkx[0:64, 64:65],
                                             kvs[0:64, HD:HD + 1], ACTF.Copy,
                                             scale=4.0)
                        nc.scalar.activation(pkx[64:128, 65:66],
                                             kvs[64:128, HD:HD + 1], ACTF.Copy,
                                             scale=4.0)
                        nc.sync.dma_start(out=ar_in[i, 0:128, b, :],
                                          in_=pkx[:])
                    ksvs = wk.tile([1, 264], BF16, tag="ksvs", bufs=2)
                    nc.vector.memset(ksvs[:], 0.0)
                    for h in range(H):
                        nc.scalar.activation(
                            ksvs[0:1, 66 * h:66 * h + 64],
                            vs_ps[0:1, 64 * h:64 * h + 64], ACTF.Copy)
                    nc.sync.dma_start(out=ar_in[0, 128:130, b, :],
                                      in_=ksvs[0:1, 0:132])
                    nc.sync.dma_start(out=ar_in[1, 128:130, b, :],
                                      in_=ksvs[0:1, 132:264])

                def q_block(j):
                    half, base = j // 2, (j % 2) * 4
                    hsl = slice(half * 128, (half + 1) * 128)
                    for bb in (base, base + 2):
                        pqs = [psA.tile([128, S], F32, tag=f"pq{i}",
                                        name=f"pq{i}", bufs=1)
                               for i in range(2)]
                        for c in range(2):
                            for i in range(2):
                                nc.tensor.matmul(
                                    pqs[i][:], lhsT=wq_t[c][:, hsl],
                                    rhs=xq_sb[bb + i][c][:],
                                    start=(c == 0), stop=(c == 1))
                        for i in range(2):
                            b = bb + i
                            nc.scalar.activation(q_sb[b][half][:], pqs[i][:],
                                                 ACTF.Identity,
                                                 bias=bq_col[half][:])
                            nc.vector.tensor_mul(qsq_sb[b][half][:],
                                                 q_sb[b][half][:],
                                                 q_sb[b][half][:])

                def ss_block(b):
                    ssp = psA.tile([4, S], F32, tag="ss", bufs=1)
                    nc.tensor.matmul(ssp[:], lhsT=i4o[:, 0:4],
                                     rhs=qsq_sb[b][0][:], start=True,
                                     stop=False)
                    nc.tensor.matmul(ssp[:], lhsT=i4o[:, 4:8],
                                     rhs=qsq_sb[b][1][:], start=False,
                                     stop=True)
                    sqb = wk.tile([4, S], F32, tag="sqb", bufs=2)
                    nc.scalar.activation(sqb[:], ssp[:],
                                         ACTF.Sqrt, scale=float(16 * N * N))
                    nc.sync.dma_start(out=sq4N_all[4 * b:4 * b + 4, :],
                                      in_=sqb[:])

                # Q-projection blocks are interleaved between VK chunks so
                # the PE fills the gaps left by the k-norm chains
                for sb in range(4):
                    for b in range(B):
                        kv_proj(b, sb)
                        if sb == 3 and b > 0:
                            kvs_phase(b - 1)
                    # chunk sb complete: one fat DMA out, then its AllGather
                    nc.sync.dma_start(out=vm_loc[sb][:], in_=vm_sb[sb][:])
                    nc.gpsimd.collective_compute(
                        "AllGather", ALU.bypass, ins=[vm_loc[sb].opt()],
                        outs=[vm_all[sb].opt()], replica_groups=RG)
                    q_block(sb)
                    if sb == 2:
                        for b in range(4):
                            ss_block(b)
                kvs_phase(B - 1)
                for b in range(4, 8):
                    ss_block(b)

            nc.gpsimd.collective_compute(
                "AllReduce", ALU.add, ins=[ar_in.opt()],
                outs=[ar_out.opt()], replica_groups=RG)

            for m in range(16):
                nc.sync.dma_start(out=adj_sb[m][:], in_=adjt[m])

            # ============ GCN (DoubleRow fp8) + attention epilogue =========
            with tc.tile_pool(name="psD", bufs=1, space="PSUM") as psD:
                prr = psD.tile([128, S], F32, tag="pbc", name="prr")
                nc.tensor.matmul(prr[:], lhsT=orr[0:1, 0:128],
                                 rhs=orr[0:1, S:2 * S], start=True, stop=True)
                nc.scalar.activation(rrs_bc[:], prr[:], ACTF.Copy)
                pg = [psD.tile([128, S], F32, tag=f"g{p}", name=f"g{p}")
                      for p in range(4)]
                if USE_DOUBLE_ROW:
                    for jp in range(2):
                        for peer in range(NCORES):
                            vmt = wk.tile([128, 2 * B * D], FP8, tag="vml",
                                          bufs=3)
                            nc.sync.dma_start(out=vmt[:, 0:512],
                                              in_=vm_all[2 * jp][peer])
                            nc.sync.dma_start(out=vmt[:, 512:1024],
                                              in_=vm_all[2 * jp + 1][peer])
                            vmv = vmt[:].rearrange("p (o f) -> p o f", o=2)
                            adv = adj_sb[peer * 2 + jp][:].rearrange(
                                "p (o f) -> p o f", o=2)
                            for p in range(4):
                                nc.tensor.matmul(
                                    pg[p][:],
                                    lhsT=vmv[:, :, 128 * p:128 * (p + 1)],
                                    rhs=adv[:],
                                    start=(jp == 0 and peer == 0),
                                    stop=(jp == 1 and peer == 7),
                                    perf_mode=PERF.DoubleRow)
                else:
                    for j in range(4):
                        for peer in range(NCORES):
                            vmt = wk.tile([128, B * D], FP8, tag="vml", bufs=3)
                            nc.sync.dma_start(out=vmt[:], in_=vm_all[j][peer])
                            adv = adj_sb[peer * 2 + j // 2][:].rearrange(
                                "p (o f) -> p o f", o=2)
                            for p in range(4):
                                nc.tensor.matmul(
                                    pg[p][:],
                                    lhsT=vmt[:, 128 * p:128 * (p + 1)],
                                    rhs=adv[:, j % 2, :],
                                    start=(j == 0 and peer == 0),
                                    stop=(j == 3 and peer == 7))

                # ---- attention epilogue (AllReduce landed during GCN) ----
                kpx_all = [pp.tile([128, B * 66], BF16, tag=f"kpxa{i}",
                                   name=f"kpxa{i}") for i in range(2)]
                for i in range(2):
                    nc.sync.dma_start(out=kpx_all[i][:],
                                      in_=ar_out[i, 0:128, :, :])
                for b in range(B):
                    for i in range(2):
                        pden = psD.tile([2, S], F32, tag="pden", bufs=2)
                        nc.tensor.matmul(
                            pden[:],
                            lhsT=kpx_all[i][:, 66 * b + 64:66 * b + 66],
                            rhs=q_sb[b][i][:], start=True, stop=True)
                        tt = wk.tile([2, S], F32, tag="tt", bufs=2)
                        nc.scalar.activation(tt[:], pden[:], ACTF.Copy)
                        nc.sync.dma_start(
                            out=t4_all[4 * b + 2 * i:4 * b + 2 * i + 2, :],
                            in_=tt[:])
                vspf = wk.tile([4, B * D], BF16, tag="vspf", bufs=1)
                for i in range(2):
                    nc.sync.dma_start(out=vspf[2 * i:2 * i + 2, :],
                                      in_=ar_out[i, 128:130, :, 0:64])
                nc.scalar.activation(vsp_all[:], vspf[:], ACTF.Copy,
                                     scale=float(1.0 / (4 * N)))
                nc.vector.tensor_add(u4_all[:], t4_all[:], sq4N_all[:])
                w1f = wk.tile([32, S], F32, tag="w1f", bufs=1)
                nc.vector.reciprocal_approx_fast(w1f[:], u4_all[:])
                nc.scalar.activation(w1_bf[:], w1f[:], ACTF.Copy)
                nc.vector.tensor_mul(w2_bf[:], w1f[:], sq4N_all[:])

                for b in range(B):
                    w2b = wk.tile([4, S], BF16, tag="w2b", bufs=2)
                    nc.sync.dma_start(out=w2b[:],
                                      in_=w2_bf[4 * b:4 * b + 4, :])
                    qs_t = []
                    for i in range(2):
                        pbc = psD.tile([128, S], F32, tag="pbc", bufs=1)
                        nc.tensor.matmul(pbc[:], lhsT=ibcb_all[:, (2 * b + i) * 128:(2 * b + i + 1) * 128],
                                         rhs=w1_bf[:], start=True, stop=True)
                        qs = wk.tile([128, S], BF16, tag=f"qs{i}", bufs=2)
                        nc.vector.tensor_mul(qs[:], q_sb[b][i][:], pbc[:])
                        qs_t.append(qs)
                    pat = psD.tile([D, S], F32, tag="pat", bufs=1)
                    nc.tensor.matmul(pat[:],
                                     lhsT=kpx_all[0][:, 66 * b:66 * b + 64],
                                     rhs=qs_t[0][:], start=True, stop=False)
                    nc.tensor.matmul(pat[:],
                                     lhsT=kpx_all[1][:, 66 * b:66 * b + 64],
                                     rhs=qs_t[1][:], start=False, stop=False)
                    nc.tensor.matmul(pat[:],
                                     lhsT=vsp_all[:, b * D:(b + 1) * D],
                                     rhs=w2b[:], start=False, stop=True)
                    nc.scalar.activation(
                        attn_sb[b // 2][(b % 2) * D:(b % 2 + 1) * D, :],
                        pat[:], ACTF.Copy)

                for p in range(4):
                    gt = wk.tile([128, S], F32, tag="gt", bufs=2)
                    nc.vector.tensor_mul(gt[:], pg[p][:], rrs_bc[:])
                    ot = wk.tile([128, S], F32, tag="ot", bufs=2)
                    nc.vector.tensor_add(ot[:], gt[:], attn_sb[p][:])
                    nc.sync.dma_start(out=out[2 * p], in_=ot[0:D, :])
                    nc.sync.dma_start(out=out[2 * p + 1], in_=ot[D:128, :])
    nc.compile()
    return nc


def _prep_inputs(query_input, source_input, adj, Wq_w, Wq_b, Wk_w, Wk_b,
                 Wv_w, Wv_b):
    bf16 = ml_dtypes.bfloat16
    fp8 = ml_dtypes.float8_e4m3fn
    xq_np = np.asarray(query_input, dtype=np.float32)
    xs_np = np.asarray(source_input, dtype=np.float32)
    adj_np = np.asarray(adj, dtype=np.float32)

    adjT = np.ascontiguousarray(adj_np.T)
    np.fill_diagonal(adjT, adjT.diagonal() + 1.0)
    adjT_f8 = adjT.astype(fp8)
    rrs_full = (0.25 / (adj_np.sum(axis=1) + 1.0)).astype(np.float32)

    wkv_np = np.concatenate([np.asarray(Wk_w, np.float32).T,
                             np.asarray(Wv_w, np.float32).T], axis=1)
    wkv_np = np.ascontiguousarray(wkv_np).astype(bf16).reshape(2, 128, 2 * HD)
    bkv_np = np.concatenate([np.asarray(Wk_b, np.float32),
                             np.asarray(Wv_b, np.float32)])
    bkv_np = bkv_np.astype(bf16).reshape(1, 2 * HD)
    wqt = np.ascontiguousarray(np.asarray(Wq_w, np.float32).T)
    wqt = wqt.astype(bf16).reshape(2, 128, HD)
    bqc = np.asarray(Wq_b, np.float32).reshape(2, 128, 1)

    i4o, ibcb = _indicators()
    in_maps = []
    for i in range(NCORES):
        sl = slice(i * S, (i + 1) * S)
        # [4096, S] -> [peer, jp, o, ki, n] -> [peer, jp, ki, o, n]
        a = adjT_f8[:, sl].reshape(8, 2, 2, 128, S)
        a = np.ascontiguousarray(a.transpose(0, 1, 3, 2, 4))
        in_maps.append({
            "xq": np.ascontiguousarray(xq_np[:, :, sl]).astype(bf16)
                  .reshape(B, 2, 128, S),
            "xs": np.ascontiguousarray(xs_np[:, :, sl]).astype(bf16)
                  .reshape(B, 2, 128, S),
            "adjt": a.reshape(16, 128, 2 * S),
            "wkv": wkv_np, "bkv": bkv_np, "wqt": wqt, "bqc": bqc,
            "i4o_in": i4o.astype(bf16),
            "ibcb_in": ibcb.astype(bf16),
            "orr_in": np.concatenate(
                [np.ones((1, S), np.float32),
                 rrs_full[sl].astype(np.float32).reshape(1, S)],
                axis=1).astype(bf16),
        })
    return in_maps


def kernel(**inputs):
    if "nc" not in _CACHE:
        _CACHE["nc"] = _build()
    nc = _CACHE["nc"]
    in_maps = _prep_inputs(**inputs)
    # run twice and keep the second result: the very first execution of a
    # freshly loaded NEFF was once observed to produce degraded numerics
    # (collective warm-up); the re-run is cheap and deterministic
    run_bass_kernel_spmd(nc, in_maps, list(range(NCORES)))
    res = run_bass_kernel_spmd(nc, in_maps, list(range(NCORES)))
    full = np.empty((B, D, N), np.float32)
    for i in range(NCORES):
        full[:, :, i * S:(i + 1) * S] = res.results[i]["out"]
    return full
